# revision 33
# baseline (speedup 1.0000x reference)
"""Causal self-attention (B=8, T=1024, D=2048, H=16) on 8 NeuronCores.

Data-parallel over the batch dim: core i handles batch element i end-to-end
(QKV proj -> causal attention -> out proj). No collectives.

Layout: everything runs on transposed activations. The host feeds x[b].T
([D, T]) in fp16; Q/K are produced d-major ([Dh, T]), V token-major, and the
output projection emits y.T which the host transposes back. Every contraction
stays on the partition dim with zero on-device transposes.

v2 over the 772us baseline:
- fp16 operands everywhere (f32 PSUM accumulation): same PE rate as f32r but
  halves DMA/SBUF and removes the fp32 HIGH/LOW two-pass broadcast matmuls.
- q-chunks of 256 (was 512): causal waste drops from 12/16 to 10/16 tiles.
- score k-tiles are computed in pairs into one [128,512] PSUM bank so each
  exp ACTIVATE covers 2 tiles (ACT was the inner-loop rate limiter), and the
  diagonal pair is masked by a single 2D-pattern affine_select.
- softmax denominator via an all-ones [128,128] stationary matmul: lands
  already partition-broadcast in PSUM, so normalize is reciprocal+mul on the
  DVE. Kills the old per-chunk fp32 K=1 broadcast matmul + scalar copy.
- V bias added on the DVE during the PSUM->SBUF copy (bias pre-broadcast to
  [128, 2048] once at startup); kills 32 K=1 bias matmuls.
- attention output stays resident in SBUF (fp16) -> no DRAM round trip
  between attention and the output projection.
"""

import math

import numpy as np

B, T, D = 8, 1024, 2048
H = 16
DH = D // H  # 128
NCT = D // 128  # 16 c-tiles
NJQ = T // 256  # 4 q-chunks of 256 for attention
SCALE = 1.0 / math.sqrt(DH)
N_CORES = 8

_CACHE = {}


def _build():
    import concourse.bacc as bacc
    import concourse.mybir as mybir
    import concourse.tile as tile

    f32 = mybir.dt.float32
    f16 = mybir.dt.float16
    Exp = mybir.ActivationFunctionType.Exp

    nc = bacc.Bacc(None, target_bir_lowering=False)

    # biases come in pre-transposed/converted from the host so their DMAs are
    # contiguous (the strided 4-byte rearrange loads took ~15us on the
    # cold queues and gated the whole prologue)
    # NOTE: the V-bias never touches the device: softmax weights sum to 1, so
    # y = sum_k a_k (V_k + b_v) = (sum_k a_k V_k) + b_v, and b_v folds into
    # b_proj_eff = b_proj + b_v @ w_proj on the host. b_qkv_t here only
    # carries the q/k per-head biases (columns 0..2*NCT-1 are used).
    xT = nc.declare_dram_parameter("xT", [D, T], f16, isOutput=False)
    w_qkv = nc.declare_dram_parameter("w_qkv", [D, 3 * D], f16, isOutput=False)
    b_qkv_t = nc.declare_dram_parameter("b_qkv_t", [128, 3 * D // 128], f32, isOutput=False)
    w_proj = nc.declare_dram_parameter("w_proj", [D, D], f16, isOutput=False)
    b_proj_t = nc.declare_dram_parameter("b_proj_t", [128, D // 128], f32, isOutput=False)
    outT = nc.declare_dram_parameter("outT", [D, T], f32, isOutput=True)

    with tile.TileContext(nc) as tc:
        with (
            tc.tile_pool(name="xbig", bufs=1) as pool_xbig,
            tc.tile_pool(name="vbig", bufs=1) as pool_vbig,
            tc.tile_pool(name="aobig", bufs=1) as pool_aobig,
            tc.tile_pool(name="qk", bufs=5) as pool_qk,
            tc.tile_pool(name="e", bufs=7) as pool_e,
            tc.tile_pool(name="esum", bufs=6) as pool_esum,
            tc.tile_pool(name="w512", bufs=28) as pool_w512,
            tc.tile_pool(name="wbig", bufs=10) as pool_wbig,
            tc.tile_pool(name="outp", bufs=3) as pool_out,
            tc.tile_pool(name="den", bufs=4) as pool_den,
            tc.tile_pool(name="misc", bufs=1) as pool_misc,
        ):
            # ---- the very first DMAs: what the first V matmuls need, spread
            # across DIFFERENT engines' DMA queues so they wake in parallel
            # (each engine has its own queues + completion semaphores; the
            # sync queue historically woke ~6us later than the first-used
            # ones, and the tile scheduler can't reorder across engines). ----
            xT_t = []
            w_fc0 = []
            t_ = pool_xbig.tile([128, T], f16, name="xT_t", tag="xbig", bufs=NCT)
            for q4 in range(4):
                # sync's FIRST descriptors ride the fast-waking queue (~2.5us
                # vs ~8.4us for the others), so give it the earliest-needed
                # quarters
                eng = (nc.sync, nc.sync, nc.sync, nc.sync)[q4]
                eng.dma_start(
                    t_[:, q4 * 256 : (q4 + 1) * 256],
                    xT[0:128, q4 * 256 : (q4 + 1) * 256],
                )
            xT_t.append(t_)
            w_t = pool_w512.tile([128, 512], f16, name="w_fc0", tag="w512")
            nc.sync.dma_start(w_t[:], w_qkv[0:128, 2 * D : 2 * D + 512])
            w_fc0.append(w_t)
            w_t = pool_w512.tile([128, 512], f16, name="w_fc0", tag="w512")
            nc.gpsimd.dma_start(w_t[:], w_qkv[128:256, 2 * D : 2 * D + 512])
            w_fc0.append(w_t)

            # ---- constants / biases ----
            # memset can't target f16 reliably; stage via f32 + DVE copy
            ones_sq_f = pool_misc.tile([128, 128], f32, tag="ones_sq_f")
            nc.vector.memset(ones_sq_f[:], 1.0)
            ones_sq = pool_misc.tile([128, 128], f16, tag="ones_sq")
            nc.vector.tensor_copy(ones_sq[:], ones_sq_f[:])

            # b_qkv as [128, 48]: column j = feature-tile j (f = j*128 + p)
            bqkv_sb = pool_misc.tile([128, 3 * D // 128], f32, tag="bqkv")
            nc.sync.dma_start(bqkv_sb[:], b_qkv_t[:, :])
            bproj_sb = pool_misc.tile([128, D // 128], f32, tag="bproj")
            nc.sync.dma_start(bproj_sb[:], b_proj_t[:, :])

            # ---- load x.T resident: 16 tiles [128, 1024] fp16. fc=0 weight
            # DMAs interleave ahead of each xT tile; xT in halves so the
            # first matmuls can start as soon as the queues warm up. ----
            t_ = pool_xbig.tile([128, T], f16, name="xT_t", tag="xbig", bufs=NCT)
            for q4 in range(4):
                eng = (nc.scalar, nc.gpsimd, nc.scalar, nc.gpsimd)[q4]
                eng.dma_start(
                    t_[:, q4 * 256 : (q4 + 1) * 256],
                    xT[128:256, q4 * 256 : (q4 + 1) * 256],
                )
            xT_t.append(t_)
            for ct in range(2, NCT):
                w_t = pool_w512.tile([128, 512], f16, name="w_fc0", tag="w512")
                nc.sync.dma_start(
                    w_t[:], w_qkv[ct * 128 : (ct + 1) * 128, 2 * D : 2 * D + 512]
                )
                w_fc0.append(w_t)
                t_ = pool_xbig.tile([128, T], f16, name="xT_t", tag="xbig", bufs=NCT)
                for half in range(2):
                    nc.sync.dma_start(
                        t_[:, half * 512 : (half + 1) * 512],
                        xT[ct * 128 : (ct + 1) * 128, half * 512 : (half + 1) * 512],
                    )
                xT_t.append(t_)

            # pre-issue QK weight DMAs for the first two heads so the sync
            # engine isn't stuck behind phase 1's ring waits at the
            # phase 1 -> phase 2 boundary
            def fetch_qk_weights(h, eng=None):
                tiles = {}
                for s, base in (("q", 0), ("k", D)):
                    w_halves = []
                    for hf in range(2):
                        w_t = pool_wbig.tile(
                            [128, NCT // 2, 128], f16, name="w_t", tag="wbig"
                        )
                        (eng or nc.sync).dma_start(
                            w_t[:],
                            w_qkv[
                                hf * (D // 2) : (hf + 1) * (D // 2),
                                base + h * 128 : base + (h + 1) * 128,
                            ].rearrange("(n p) f -> p n f", p=128),
                        )
                        w_halves.append(w_t)
                    tiles[s] = w_halves
                return tiles

            # prefetch the first two heads' QK weights on the scalar engine's
            # queues: idle early, own semaphores, so neither the scheduler nor
            # phase 1's ring-gated weight stream can delay them
            whead = {
                0: fetch_qk_weights(0, nc.scalar),
                1: fetch_qk_weights(1, nc.scalar),
            }

            # ---- phase 1: V for all heads, token-major [128, 8, 2048] ----
            V_sb = pool_vbig.tile([128, T // 128, D], f16, tag="vbig")
            with tc.tile_pool(name="p1psum", bufs=8, space="PSUM") as pool_p1:
                for fc in range(D // 512):
                    # the LAST fc runs in two 4-bank waves so half the PSUM
                    # banks free ~7us before phase 1 ends and head 0's QK
                    # matmuls aren't stuck waiting for banks
                    last = fc == D // 512 - 1
                    tt_waves = ([0, 1, 2, 3], [4, 5], [6, 7]) if last else (list(range(8)),)
                    w_fc = []
                    for ct in range(NCT):
                        if fc == 0:
                            w_t = w_fc0[ct]
                        else:
                            w_t = pool_w512.tile(
                                [128, 512], f16, name="w_t", tag="w512"
                            )
                            nc.sync.dma_start(
                                w_t[:],
                                w_qkv[
                                    ct * 128 : (ct + 1) * 128,
                                    2 * D + fc * 512 : 2 * D + (fc + 1) * 512,
                                ],
                            )
                        w_fc.append(w_t)
                    for wave in tt_waves:
                        ps_v = {
                            tt: pool_p1.tile([128, 512], f32, name="vps", tag="vps")
                            for tt in wave
                        }
                        for ct in range(NCT):
                            for tt in wave:
                                nc.tensor.matmul(
                                    ps_v[tt][:],
                                    xT_t[ct][:, tt * 128 : (tt + 1) * 128],
                                    w_fc[ct][:],
                                    start=(ct == 0),
                                    stop=(ct == NCT - 1),
                                )
                        for tt in wave:
                            # PSUM -> SBUF fp16 on the scalar engine (idle in
                            # phase 1); V-bias is folded into b_proj on host
                            nc.scalar.copy(
                                V_sb[:, tt, fc * 512 : (fc + 1) * 512],
                                ps_v[tt][:],
                            )

            # ---- phase 2: per-head attention; ao stays resident in SBUF ----
            ao_t = [
                pool_aobig.tile([128, T], f16, name="ao_t", tag="aobig", bufs=NCT)
                for _ in range(H)
            ]
            with (
                tc.tile_pool(name="sps", bufs=4, space="PSUM") as pool_s,
                tc.tile_pool(name="qaps", bufs=2, space="PSUM") as pool_qa,
                tc.tile_pool(name="ydps", bufs=2, space="PSUM") as pool_yd,
            ):
                for h in range(H):
                    # 2a: Q^T and K^T for head h, d-major [128, 1024] fp16
                    wt = whead.pop(h) if h in whead else fetch_qk_weights(h)
                    if h + 2 < H and (h + 2) not in whead:
                        whead[h + 2] = fetch_qk_weights(h + 2)
                    qk = {}
                    for s, btile in (("q", h), ("k", NCT + h)):
                        sb = pool_qk.tile([128, T], f16, tag="qk")
                        w_halves = wt[s]
                        for jc in range(2):
                            ps = pool_qa.tile([128, 512], f32, name="qkps", tag="qa")
                            for ct in range(NCT):
                                nc.tensor.matmul(
                                    ps[:],
                                    w_halves[ct // 8][:, ct % 8, :],
                                    xT_t[ct][:, jc * 512 : (jc + 1) * 512],
                                    start=(ct == 0),
                                    stop=(ct == NCT - 1),
                                )
                            nc.vector.tensor_scalar_add(
                                sb[:, jc * 512 : (jc + 1) * 512],
                                ps[:],
                                bqkv_sb[:, btile : btile + 1],
                            )
                        qk[s] = sb

                    # 2b: causal attention, scores transposed [k, q], q-chunks
                    # of 256, k-tiles processed in pairs sharing a PSUM bank
                    for jq in range(NJQ):
                        npair = jq + 1  # k-tile pairs 0 .. jq (rest masked)
                        esum = []
                        ps_yd = pool_yd.tile([128, 2, 256], f32, tag="yd")
                        ps_y = ps_yd[:, 0, :]
                        ps_d = ps_yd[:, 1, :]
                        es = []
                        for p in range(npair):
                            # one accumulation group spans the bank: the 2nd
                            # matmul writes pending-zero bytes (= overwrite)
                            ps_s = pool_s.tile([128, 2, 256], f32, tag="mm512")
                            for i in range(2):
                                nc.tensor.matmul(
                                    ps_s[:, i, :],
                                    qk["k"][:, (2 * p + i) * 128 : (2 * p + i + 1) * 128],
                                    qk["q"][:, jq * 256 : (jq + 1) * 256],
                                    start=(i == 0),
                                    stop=(i == 1),
                                )
                            e_t = pool_e.tile([128, 2, 256], f16, tag="e")
                            nc.scalar.activation(e_t[:], ps_s[:], Exp, scale=SCALE)
                            if p == npair - 1:
                                # diagonal pair: keep where k <= q, i.e.
                                # q - 128*i - part >= 0 over free dims (i, q)
                                nc.gpsimd.affine_select(
                                    out=e_t[:],
                                    in_=e_t[:],
                                    compare_op=mybir.AluOpType.is_ge,
                                    fill=0.0,
                                    base=0,
                                    pattern=[[-128, 2], [1, 256]],
                                    channel_multiplier=-1,
                                )
                            else:
                                # off-diagonal pair: pre-sum its two e tiles
                                # on gpsimd so the denominator takes ONE
                                # matmul per pair instead of two
                                f_t = pool_esum.tile([128, 256], f16, tag="es")
                                nc.gpsimd.tensor_add(
                                    f_t[:], e_t[:, 0, :], e_t[:, 1, :]
                                )
                                esum.append(f_t)
                            es.append(e_t)
                            # attnV + denominator for the PREVIOUS pair keep
                            # the PE busy while ACT runs this pair's exp
                            # ps_y and ps_d share one bank = ONE accumulation
                            # group: start only on the very first matmul, stop
                            # only on the very last; intermediate writes to
                            # pending-zero bytes overwrite (first-write
                            # semantics per byte).
                            if p > 0:
                                pe = es[p - 1]
                                for i in range(2):
                                    ki = 2 * (p - 1) + i
                                    nc.tensor.matmul(
                                        ps_y,
                                        V_sb[:, ki, h * 128 : (h + 1) * 128],
                                        pe[:, i, :],
                                        start=(ki == 0),
                                        stop=False,
                                    )
                                nc.tensor.matmul(
                                    ps_d,
                                    ones_sq[:],
                                    esum[p - 1][:],
                                    start=False,
                                    stop=False,
                                )
                        pe = es[npair - 1]
                        for i in range(2):
                            ki = 2 * (npair - 1) + i
                            nc.tensor.matmul(
                                ps_y,
                                V_sb[:, ki, h * 128 : (h + 1) * 128],
                                pe[:, i, :],
                                start=(ki == 0),
                                stop=False,
                            )
                            # diagonal pair keeps per-k-tile den matmuls (the
                            # mask + pair-sum chain would add gpsimd latency
                            # right at the end of the chunk)
                            nc.tensor.matmul(
                                ps_d,
                                ones_sq[:],
                                pe[:, i, :],
                                start=False,
                                stop=(i == 1),
                            )
                        # denominator arrives already broadcast across
                        # partitions; normalize fully on the DVE
                        inv_d = pool_den.tile([128, 256], f32, tag="invden")
                        nc.vector.reciprocal_approx_fast(out=inv_d[:], in_=ps_d)
                        nc.vector.tensor_mul(
                            ao_t[h][:, jq * 256 : (jq + 1) * 256], ps_y, inv_d[:]
                        )

            # ---- phase 3: output projection from resident ao, emitted
            # transposed ----
            with tc.tile_pool(name="p3psum", bufs=4, space="PSUM") as pool_p3:
                for dt in range(D // 128):
                    wp_halves = []
                    for hf in range(2):
                        wp_t = pool_wbig.tile(
                            [128, NCT // 2, 128], f16, name="wp_t", tag="wbig"
                        )
                        nc.sync.dma_start(
                            wp_t[:],
                            w_proj[
                                hf * (D // 2) : (hf + 1) * (D // 2),
                                dt * 128 : (dt + 1) * 128,
                            ].rearrange("(n p) f -> p n f", p=128),
                        )
                        wp_halves.append(wp_t)
                    for jc in range(2):
                        ps = pool_p3.tile([128, 512], f32, tag="mm512")
                        for ct in range(NCT):
                            nc.tensor.matmul(
                                ps[:],
                                wp_halves[ct // 8][:, ct % 8, :],
                                ao_t[ct][:, jc * 512 : (jc + 1) * 512],
                                start=(ct == 0),
                                stop=(ct == NCT - 1),
                            )
                        o_t = pool_out.tile([128, 512], f32, tag="outp")
                        nc.vector.tensor_scalar_add(
                            o_t[:], ps[:], bproj_sb[:, dt : dt + 1]
                        )
                        nc.sync.dma_start(
                            outT[dt * 128 : (dt + 1) * 128, jc * 512 : (jc + 1) * 512],
                            o_t[:],
                        )

    nc.compile()
    return nc


def _get_nc():
    if "nc" not in _CACHE:
        _CACHE["nc"] = _build()
    return _CACHE["nc"]


def kernel(x, w_qkv, b_qkv, w_proj, b_proj, _trace=False, _trace_kwargs=None):
    from concourse.bass_utils import run_bass_kernel_spmd

    x = np.asarray(x, dtype=np.float32)
    w_qkv16 = np.asarray(w_qkv, dtype=np.float16)
    b_qkv = np.asarray(b_qkv, dtype=np.float32)
    w_proj16 = np.asarray(w_proj, dtype=np.float16)
    b_proj = np.asarray(b_proj, dtype=np.float32)

    nc = _get_nc()
    # biases pre-transposed to the on-chip layouts (see _build); the V-bias
    # is folded into the projection bias: y = sum_k a_k (V_k + b_v) =
    # (sum a V) + b_v  because softmax weights sum to 1
    b_qkv_t = np.ascontiguousarray(b_qkv.reshape(3 * D // 128, 128).T)
    b_proj_eff = b_proj + b_qkv[2 * D : 3 * D] @ w_proj.astype(np.float32)
    b_proj_t = np.ascontiguousarray(b_proj_eff.reshape(D // 128, 128).T)
    in_maps = []
    for i in range(N_CORES):
        in_maps.append(
            {
                "xT": np.ascontiguousarray(x[i].T.astype(np.float16)),
                "w_qkv": w_qkv16,
                "b_qkv_t": b_qkv_t,
                "w_proj": w_proj16,
                "b_proj_t": b_proj_t,
            }
        )
    res = run_bass_kernel_spmd(
        nc,
        in_maps,
        list(range(N_CORES)),
        trace=_trace,
        **(_trace_kwargs or {}),
    )
    y = np.stack(
        [np.ascontiguousarray(res.results[i]["outT"].T) for i in range(N_CORES)]
    )
    if _trace:
        _CACHE["last_result"] = res
    return y
